# revision 1
# baseline (speedup 1.0000x reference)
"""Trainium2 Bass kernel for nn_LocalSwarmAggregator (sliding-window causal MHA).

Reference computation (fp32):
    q,k,v = x@Wq+bq, x@Wk+bk, x@Wv+bv          # [B,N,D] -> per-head [B,H,N,64]
    logits = q k^T / 8 + band_mask              # causal + 256-window
    out = softmax(logits) v                     # [B,H,N,64]
    y = concat_heads(out) @ Wo + bo             # [B,N,D]

Sharding over 8 cores: core c handles batch c//4 and heads 4*(c%4)..4*(c%4)+3
(tensor-parallel on the head dim of Wq/Wk/Wv and the row dim of Wo).  Each
core computes a partial y for its batch; the host sums the 4 partials per
batch and adds bo.  No cross-device communication.

Per-core kernel layout (all fp32 storage):
  - x^T [D,N] built on-chip via PE transposes (projections contract over D).
  - q^T,k^T [256,N] head-pair-stacked; v^T transposed again to v natural,
    augmented with a ones column (v_aug) so the attention-weight row sums
    come out of the AV matmul for free.
  - S^T tiles [128 keys, up to 384 queries]: for key tile kt the only
    queries attending are 128*kt .. 128*kt+383, and the valid band within
    the tile is r <= c <= r+256 for every kt -> one constant 0/1 mask.
  - P^T = exp(S^T/8) * mask01 (no row-max subtraction needed: logits are
    O(6) so exp is safe in fp32).
  - AV: out^T_aug[65, q] accumulated over kt in PSUM (has_written gives
    overwrite-then-accumulate per element), row 64 = softmax denominators.
  - normalize via reciprocal + gpsimd partition_broadcast, then the output
    projection contracts head pairs (K=128) against Wo row-pairs.
"""

import os
from contextlib import ExitStack

import numpy as np

import concourse.bass as bass
import concourse.mybir as mybir
import concourse.tile as tile
from concourse import bacc
from concourse.bass_utils import run_bass_kernel_spmd
from concourse.masks import make_identity

F32 = mybir.dt.float32
N = 2048
D = 1024
HD = 64
WIN = 256
NPAIR = 2  # head pairs per core (4 heads)
NSEQT = N // 128  # 16
NDCH = D // 128  # 8
NKT = N // 128  # 16 key tiles
SPAN = 384  # max query span per S^T key tile
QG = 512  # AV / projection query group size
NQG = N // QG  # 4
SCALE = 1.0 / np.sqrt(HD)

# matmul compute dtype: float32r runs 4x faster on the PE than float32
# (single-pass reduced-precision mode vs 2-pass exact fp32).
_MM_DT_NAME = os.environ.get("BASS_MM_DT", "float32r")
MM_DT = getattr(mybir.dt, _MM_DT_NAME)

Exp = mybir.ActivationFunctionType.Exp
IS_GE = mybir.AluOpType.is_ge


def _av_slices(kt):
    """For key tile kt return [(g, lo, hi, plo)]: query-group g consumes
    P^T[kt][:, lo:hi] into psum columns plo:plo+(hi-lo)."""
    span = min(SPAN, N - 128 * kt)
    out = []
    for g in range(NQG):
        lo = max(0, QG * g - 128 * kt)
        hi = min(span, QG * g + QG - 128 * kt)
        if lo < hi:
            out.append((g, lo, hi, 128 * kt + lo - QG * g))
    return out


def _group_kts(g):
    """Key tiles contributing to query group g (ordered)."""
    return [kt for kt in range(NKT) if any(s[0] == g for s in _av_slices(kt))]


def _emit(ctx: ExitStack, tc: tile.TileContext, aps, mm_dt):
    nc = tc.nc
    x, wq, wk, wv, wo, bq, bk, bv, out = aps
    MDT = mm_dt

    def fr(ap):
        return ap

    consts = ctx.enter_context(tc.tile_pool(name="consts", bufs=1))
    persist = ctx.enter_context(tc.tile_pool(name="persist", bufs=1))

    ident_f = consts.tile([128, 128], F32, tag="ident_f")
    make_identity(nc, ident_f)
    ident = consts.tile([128, 128], MDT, tag="ident")
    nc.vector.tensor_copy(ident, ident_f)

    # 0/1 band mask: valid iff r <= c <= r + WIN  (keys on partitions,
    # query offset on free dim)
    mask = consts.tile([128, SPAN], F32, tag="mask")
    mask2 = consts.tile([128, 2, SPAN], F32, tag="mask2")
    nc.gpsimd.memset(mask, 1.0)
    nc.gpsimd.affine_select(
        out=mask, in_=mask, compare_op=IS_GE, fill=0.0,
        base=0, pattern=[[1, SPAN]], channel_multiplier=-1,
    )  # keep c - r >= 0
    nc.gpsimd.affine_select(
        out=mask, in_=mask, compare_op=IS_GE, fill=0.0,
        base=WIN, pattern=[[-1, SPAN]], channel_multiplier=1,
    )  # keep r - c + WIN >= 0
    nc.gpsimd.tensor_copy(mask2[:, 0, :], mask)
    nc.gpsimd.tensor_copy(mask2[:, 1, :], mask)

    # zero / ones helpers (DVE-written so fp32r versions count as rounded)
    zf = consts.tile([1, QG], F32, tag="zf")
    nc.vector.memset(zf, 0.0)
    onesf = consts.tile([128, 1], F32, tag="onesf")
    nc.vector.memset(onesf, 1.0)
    zcol = consts.tile([1, 65], MDT, tag="zcol")
    nc.vector.tensor_copy(zcol, zf[:, 0:65])
    zrow = consts.tile([1, QG], MDT, tag="zrow")
    nc.vector.tensor_copy(zrow, zf)

    # persistent intermediates
    qT = persist.tile([128, NPAIR, N], MDT, tag="qT")  # 16KB
    kT = persist.tile([128, NPAIR, N], MDT, tag="kT")  # 16KB
    vaug = [persist.tile([128, NKT, 2, HD + 1], MDT, tag=f"vaug{p}",
                         name=f"vaug{p}")
            for p in range(NPAIR)]  # 8.3KB each
    U2 = persist.tile([128, NPAIR, N], MDT, tag="U2")  # 16KB

    # ---------------- phase A: x^T + QKV projections + v_aug ----------------
    with ExitStack() as pha:
        xn_pool = pha.enter_context(tc.tile_pool(name="xn", bufs=4))
        xt_pool = pha.enter_context(tc.tile_pool(name="xt", bufs=3))
        vt_pool = pha.enter_context(tc.tile_pool(name="vt", bufs=1))
        psA = pha.enter_context(tc.tile_pool(name="psA", bufs=4, space="PSUM"))
        psQ = pha.enter_context(tc.tile_pool(name="psQ", bufs=4, space="PSUM"))

        vT = vt_pool.tile([128, NPAIR, N], MDT, tag="vT")
        w_sb = {}
        b_sb = {}

        for g in range(NQG):
            # x^T for this query group, built via PE transposes
            xTg = xt_pool.tile([128, NDCH, QG], MDT, tag="xTg")
            for si in range(4):
                s = 4 * g + si
                xn = xn_pool.tile([128, D], MDT, tag="xn")
                nc.sync.dma_start(out=xn, in_=x[128 * s:128 * (s + 1), :])
                for dh in range(2):
                    ps = psA.tile([128, QG], MDT, tag="psA")
                    for dj in range(4):
                        d = 4 * dh + dj
                        nc.tensor.transpose(
                            ps[:, 128 * dj:128 * (dj + 1)],
                            xn[:, 128 * d:128 * (d + 1)], ident,
                        )
                    cp = nc.scalar.copy if (si + dh) % 2 == 0 \
                        else nc.vector.tensor_copy
                    cp(xTg[:, 4 * dh:4 * (dh + 1), 128 * si:128 * (si + 1)],
                       ps.rearrange("p (c q) -> p c q", q=128))

            if g == 0:
                # weights + biases (emitted after the first x tiles so x
                # wins the DMA queues; only QKV matmuls gate on these)
                for nm, wap in (("q", wq), ("k", wk), ("v", wv)):
                    t = consts.tile([128, NDCH, 2 * 128], MDT, tag=f"w{nm}",
                                    name=f"w{nm}")
                    nc.sync.dma_start(
                        out=t, in_=wap.rearrange("(c p) m -> p c m", p=128))
                    w_sb[nm] = t
                wo_sb = consts.tile([128, NPAIR, D], MDT, tag="wo")
                nc.sync.dma_start(
                    out=wo_sb, in_=wo.rearrange("(pair p) m -> p pair m", p=128))
                for nm, bap in (("q", bq), ("k", bk), ("v", bv)):
                    t = consts.tile([128, NPAIR], F32, tag=f"b{nm}",
                                    name=f"b{nm}")
                    nc.sync.dma_start(
                        out=t, in_=bap.rearrange("(pair p) -> p pair", p=128))
                    b_sb[nm] = t

            for pair in range(NPAIR):
                for nm, dstT in (("q", qT), ("k", kT), ("v", vT)):
                    psq = psQ.tile([128, QG], F32, tag="psQ")
                    for d in range(NDCH):
                        nc.tensor.matmul(
                            psq,
                            fr(w_sb[nm][:, d, 128 * pair:128 * (pair + 1)]),
                            fr(xTg[:, d, :]),
                            start=(d == 0), stop=(d == NDCH - 1),
                        )
                    nc.vector.tensor_scalar_add(
                        dstT[:, pair, QG * g:QG * (g + 1)], psq,
                        b_sb[nm][:, pair:pair + 1],
                    )

                if g == 0:
                    nc.vector.tensor_copy(
                        vaug[pair][:, :, :, HD:HD + 1],
                        onesf.broadcast_to((128, NKT, 2, 1)),
                    )
                # v natural for this query group's key tiles
                ps = psA.tile([128, QG], MDT, tag="psA")
                for j in range(4):
                    kt = 4 * g + j
                    nc.tensor.transpose(
                        ps[:, 128 * j:128 * (j + 1)],
                        vT[:, pair, 128 * kt:128 * (kt + 1)], ident,
                    )
                cp = nc.scalar.copy if (g + pair) % 2 == 0 \
                    else nc.vector.tensor_copy
                cp(
                    vaug[pair][:, 4 * g:4 * (g + 1), :, 0:HD],
                    ps.rearrange("p (j h d) -> p j h d", j=4, h=2),
                )


    # ------- phase B+C: attention (kg-pipelined) with interleaved out-proj -----
    with ExitStack() as phb:
        psS = phb.enter_context(tc.tile_pool(name="psS", bufs=2, space="PSUM"))
        psAV = phb.enter_context(tc.tile_pool(name="psAV", bufs=2, space="PSUM"))
        psO = phb.enter_context(tc.tile_pool(name="psO", bufs=2, space="PSUM"))
        pt_pool = phb.enter_context(tc.tile_pool(name="pt", bufs=12))
        rb_pool = phb.enter_context(tc.tile_pool(name="rb", bufs=3))
        ob_pool = phb.enter_context(tc.tile_pool(name="ob", bufs=5))

        first_kt = {g: _group_kts(g)[0] for g in range(NQG)}
        last_kt = {g: _group_kts(g)[-1] for g in range(NQG)}

        def outproj_group(g):
            for qt in range(4 * g, 4 * (g + 1)):
                for dh in range(2):
                    pso = psO.tile([128, QG], F32, tag="psO", name="pso")
                    for pair in range(NPAIR):
                        nc.tensor.matmul(
                            pso,
                            U2[:, pair, 128 * qt:128 * (qt + 1)],
                            wo_sb[:, pair, QG * dh:QG * (dh + 1)],
                            start=(pair == 0), stop=(pair == NPAIR - 1),
                        )
                    ob = ob_pool.tile([128, QG], F32, tag="ob")
                    nc.vector.tensor_copy(ob, pso)
                    nc.sync.dma_start(
                        out=out[128 * qt:128 * (qt + 1), QG * dh:QG * (dh + 1)],
                        in_=ob,
                    )

        def emit_av_group(pair, g, pts):
            """AV + normalization for query group g, both heads; pts maps
            (h, kt) -> (pt_tile, j)."""
            for h in range(2):
                psav = psAV.tile([65, QG], F32, tag="psAV", name="psav")
                nc.tensor.matmul(psav, zcol, zrow, start=True, stop=False)
                for kt in _group_kts(g):
                    pt, j = pts[(h, kt)]
                    lo, hi, plo = next(
                        (s[1], s[2], s[3]) for s in _av_slices(kt) if s[0] == g)
                    nc.tensor.matmul(
                        psav[:, plo:plo + (hi - lo)],
                        vaug[pair][:, kt, h, :],
                        pt[:, j, lo:hi],
                        start=False, stop=(kt == last_kt[g]),
                    )
                rt0 = rb_pool.tile([1, QG], F32, tag="rt0")
                nc.scalar.copy(rt0, psav[64:65, :])
                rtmp = rb_pool.tile([1, QG], F32, tag="rtmp")
                nc.vector.reciprocal_approx_fast(out=rtmp, in_=rt0)
                rbt = rb_pool.tile([64, QG], F32, tag="rb")
                nc.gpsimd.partition_broadcast(rbt, rtmp)
                nc.vector.tensor_mul(
                    U2[64 * h:64 * (h + 1), pair, QG * g:QG * (g + 1)],
                    psav[0:64, :], rbt,
                )
            if pair == NPAIR - 1:
                outproj_group(g)

        for pair in range(NPAIR):
            pts = {}
            for kg in range(NKT // 2):
                kts = [2 * kg, 2 * kg + 1]
                pss = [psS.tile([128, 2, QG], F32, tag="psS", name="pss")
                       for _ in range(2)]
                for j, kt in enumerate(kts):
                    q0 = 128 * kt
                    span = min(SPAN, N - q0)
                    for h in range(2):
                        hb = 64 * h
                        nc.tensor.matmul(
                            pss[h][:, j, 0:span],
                            kT[hb:hb + 64, pair, q0:q0 + 128],
                            qT[hb:hb + 64, pair, q0:q0 + span],
                            start=True, stop=True,
                        )
                for h in range(2):
                    pt = pt_pool.tile([128, 2, SPAN], MDT, tag="pt")
                    if kg < NKT // 2 - 1:
                        nc.scalar.activation(
                            pt[:, :, :], pss[h][:, :, 0:SPAN], Exp, scale=SCALE
                        )
                        nc.vector.tensor_mul(
                            pt[:, :, 0:128], pt[:, :, 0:128],
                            mask2[:, :, 0:128],
                        )
                        nc.vector.tensor_mul(
                            pt[:, :, WIN:SPAN], pt[:, :, WIN:SPAN],
                            mask2[:, :, WIN:SPAN],
                        )
                    else:
                        for j, kt in enumerate(kts):
                            span = min(SPAN, N - 128 * kt)
                            nc.scalar.activation(
                                pt[:, j, 0:span], pss[h][:, j, 0:span], Exp,
                                scale=SCALE,
                            )
                            nc.vector.tensor_mul(
                                pt[:, j, 0:128], pt[:, j, 0:128], mask[:, 0:128]
                            )
                            if span > WIN:
                                nc.vector.tensor_mul(
                                    pt[:, j, WIN:span], pt[:, j, WIN:span],
                                    mask[:, WIN:span],
                                )
                    for j, kt in enumerate(kts):
                        pts[(h, kt)] = (pt, j)
                if kg % 2 == 1:
                    emit_av_group(pair, (kg - 1) // 2, pts)


def build(mm_dt=MM_DT):
    nc = bacc.Bacc("TRN2", target_bir_lowering=False, debug=False)
    x = nc.dram_tensor("x", [N, D], mm_dt, kind="ExternalInput").ap()
    wq = nc.dram_tensor("wq", [D, 256], mm_dt, kind="ExternalInput").ap()
    wk = nc.dram_tensor("wk", [D, 256], mm_dt, kind="ExternalInput").ap()
    wv = nc.dram_tensor("wv", [D, 256], mm_dt, kind="ExternalInput").ap()
    wo = nc.dram_tensor("wo", [256, D], mm_dt, kind="ExternalInput").ap()
    bq = nc.dram_tensor("bq", [256], F32, kind="ExternalInput").ap()
    bk = nc.dram_tensor("bk", [256], F32, kind="ExternalInput").ap()
    bv = nc.dram_tensor("bv", [256], F32, kind="ExternalInput").ap()
    out = nc.dram_tensor("out", [N, D], F32, kind="ExternalOutput").ap()
    with tile.TileContext(nc) as tc, ExitStack() as ctx:
        _emit(ctx, tc, (x, wq, wk, wv, wo, bq, bk, bv, out), mm_dt)
    nc.compile()
    return nc


def shard_inputs(x, Wq, bq, Wk, bk, Wv, bv, Wo, bo):
    """Full inputs -> list of 8 per-core input maps."""
    in_maps = []
    for c in range(8):
        b, hg = c // 4, c % 4
        cs = slice(256 * hg, 256 * (hg + 1))
        in_maps.append({
            "x": np.ascontiguousarray(x[b]),
            "wq": np.ascontiguousarray(Wq[:, cs]),
            "wk": np.ascontiguousarray(Wk[:, cs]),
            "wv": np.ascontiguousarray(Wv[:, cs]),
            "wo": np.ascontiguousarray(Wo[cs, :]),
            "bq": np.ascontiguousarray(bq[cs]),
            "bk": np.ascontiguousarray(bk[cs]),
            "bv": np.ascontiguousarray(bv[cs]),
        })
    return in_maps


def assemble(results, bo):
    """8 per-core partial outputs -> full [2, N, D] output."""
    outs = [np.asarray(r["out"], dtype=np.float32) for r in results]
    full = np.empty((2, N, D), dtype=np.float32)
    for b in range(2):
        full[b] = outs[4 * b] + outs[4 * b + 1] + outs[4 * b + 2] + outs[4 * b + 3]
        full[b] += bo[None, :]
    return full


_NC_CACHE = {}


def _get_nc():
    key = _MM_DT_NAME
    if key not in _NC_CACHE:
        _NC_CACHE[key] = build()
    return _NC_CACHE[key]


def kernel(x, Wq, bq, Wk, bk, Wv, bv, Wo, bo, _trace=False):
    x, Wq, bq, Wk, bk, Wv, bv, Wo, bo = (
        np.asarray(a, dtype=np.float32)
        for a in (x, Wq, bq, Wk, bk, Wv, bv, Wo, bo)
    )
    nc = _get_nc()
    in_maps = shard_inputs(x, Wq, bq, Wk, bk, Wv, bv, Wo, bo)
    res = run_bass_kernel_spmd(nc, in_maps, core_ids=list(range(8)), trace=_trace)
    full = assemble(res.results, bo)
    if _trace:
        kernel.last_result = res
    return full



# revision 2
# speedup vs baseline: 1.1126x; 1.1126x over previous
"""Trainium2 Bass kernel for nn_LocalSwarmAggregator (sliding-window causal MHA).

Reference computation (fp32):
    q,k,v = x@Wq+bq, x@Wk+bk, x@Wv+bv          # [B,N,D] -> per-head [B,H,N,64]
    logits = q k^T / 8 + band_mask              # causal + 256-window
    out = softmax(logits) v                     # [B,H,N,64]
    y = concat_heads(out) @ Wo + bo             # [B,N,D]

Sharding over 8 cores: core c handles batch c//4 and heads 4*(c%4)..4*(c%4)+3
(tensor-parallel on the head dim of Wq/Wk/Wv and the row dim of Wo).  Each
core computes a partial y for its batch; the host sums the 4 partials per
batch and adds bo.  No cross-device communication.

Per-core kernel layout (all fp32 storage):
  - x^T [D,N] built on-chip via PE transposes (projections contract over D).
  - q^T,k^T [256,N] head-pair-stacked; v^T transposed again to v natural,
    augmented with a ones column (v_aug) so the attention-weight row sums
    come out of the AV matmul for free.
  - S^T tiles [128 keys, up to 384 queries]: for key tile kt the only
    queries attending are 128*kt .. 128*kt+383, and the valid band within
    the tile is r <= c <= r+256 for every kt -> one constant 0/1 mask.
  - P^T = exp(S^T/8) * mask01 (no row-max subtraction needed: logits are
    O(6) so exp is safe in fp32).
  - AV: out^T_aug[65, q] accumulated over kt in PSUM (has_written gives
    overwrite-then-accumulate per element), row 64 = softmax denominators.
  - normalize via reciprocal + gpsimd partition_broadcast, then the output
    projection contracts head pairs (K=128) against Wo row-pairs.
"""

import os
from contextlib import ExitStack

import numpy as np

import concourse.bass as bass
import concourse.mybir as mybir
import concourse.tile as tile
from concourse import bacc
from concourse.bass_utils import run_bass_kernel_spmd
from concourse.masks import make_identity

F32 = mybir.dt.float32
N = 2048
D = 1024
HD = 64
WIN = 256
NPAIR = 2  # head pairs per core (4 heads)
NSEQT = N // 128  # 16
NDCH = D // 128  # 8
NKT = N // 128  # 16 key tiles
SPAN = 384  # max query span per S^T key tile
QG = 512  # AV / projection query group size
NQG = N // QG  # 4
SCALE = 1.0 / np.sqrt(HD)

# matmul compute dtype: float32r runs 4x faster on the PE than float32
# (single-pass reduced-precision mode vs 2-pass exact fp32).
_MM_DT_NAME = os.environ.get("BASS_MM_DT", "float32r")
MM_DT = getattr(mybir.dt, _MM_DT_NAME)

Exp = mybir.ActivationFunctionType.Exp
IS_GE = mybir.AluOpType.is_ge


def _av_slices(kt):
    """For key tile kt return [(g, lo, hi, plo)]: query-group g consumes
    P^T[kt][:, lo:hi] into psum columns plo:plo+(hi-lo)."""
    span = min(SPAN, N - 128 * kt)
    out = []
    for g in range(NQG):
        lo = max(0, QG * g - 128 * kt)
        hi = min(span, QG * g + QG - 128 * kt)
        if lo < hi:
            out.append((g, lo, hi, 128 * kt + lo - QG * g))
    return out


def _group_kts(g):
    """Key tiles contributing to query group g (ordered)."""
    return [kt for kt in range(NKT) if any(s[0] == g for s in _av_slices(kt))]


def _emit(ctx: ExitStack, tc: tile.TileContext, aps, mm_dt):
    nc = tc.nc
    x, wq, wk, wv, wo, bq, bk, bv, out = aps
    MDT = mm_dt

    def fr(ap):
        return ap

    consts = ctx.enter_context(tc.tile_pool(name="consts", bufs=1))
    persist = ctx.enter_context(tc.tile_pool(name="persist", bufs=1))

    ident_f = consts.tile([128, 128], F32, tag="ident_f")
    make_identity(nc, ident_f)
    ident = consts.tile([128, 128], MDT, tag="ident")
    nc.vector.tensor_copy(ident, ident_f)

    # 0/1 band mask: valid iff r <= c <= r + WIN  (keys on partitions,
    # query offset on free dim)
    mask = consts.tile([128, SPAN], F32, tag="mask")
    mask2 = consts.tile([128, 2, SPAN], F32, tag="mask2")
    nc.gpsimd.memset(mask, 1.0)
    nc.gpsimd.affine_select(
        out=mask, in_=mask, compare_op=IS_GE, fill=0.0,
        base=0, pattern=[[1, SPAN]], channel_multiplier=-1,
    )  # keep c - r >= 0
    nc.gpsimd.affine_select(
        out=mask, in_=mask, compare_op=IS_GE, fill=0.0,
        base=WIN, pattern=[[-1, SPAN]], channel_multiplier=1,
    )  # keep r - c + WIN >= 0
    nc.gpsimd.tensor_copy(mask2[:, 0, :], mask)
    nc.gpsimd.tensor_copy(mask2[:, 1, :], mask)

    # zero / ones helpers (DVE-written so fp32r versions count as rounded)
    zf = consts.tile([1, QG], F32, tag="zf")
    nc.vector.memset(zf, 0.0)
    onesf = consts.tile([128, 1], F32, tag="onesf")
    nc.vector.memset(onesf, 1.0)
    zcol = consts.tile([1, 65], MDT, tag="zcol")
    nc.vector.tensor_copy(zcol, zf[:, 0:65])
    zrow = consts.tile([1, QG], MDT, tag="zrow")
    nc.vector.tensor_copy(zrow, zf)

    # persistent intermediates
    qT = persist.tile([128, NPAIR, N], MDT, tag="qT")  # 16KB
    kT = persist.tile([128, NPAIR, N], MDT, tag="kT")  # 16KB
    vaug = [persist.tile([128, NKT, 2, HD + 1], MDT, tag=f"vaug{p}",
                         name=f"vaug{p}")
            for p in range(NPAIR)]  # 8.3KB each
    U2 = persist.tile([128, NPAIR, N], MDT, tag="U2")  # 16KB

    # ---------------- phase A: x^T + QKV projections + v_aug ----------------
    with ExitStack() as pha:
        xn_pool = pha.enter_context(tc.tile_pool(name="xn", bufs=4))
        xt_pool = pha.enter_context(tc.tile_pool(name="xt", bufs=3))
        vt_pool = pha.enter_context(tc.tile_pool(name="vt", bufs=1))
        psA = pha.enter_context(tc.tile_pool(name="psA", bufs=4, space="PSUM"))
        psQ = pha.enter_context(tc.tile_pool(name="psQ", bufs=4, space="PSUM"))

        vT = vt_pool.tile([128, NPAIR, N], MDT, tag="vT")
        w_sb = {}
        b_sb = {}

        for g in range(NQG):
            # x^T for this query group, built via PE transposes
            xTg = xt_pool.tile([128, NDCH, QG], MDT, tag="xTg")
            for si in range(4):
                s = 4 * g + si
                xn = xn_pool.tile([128, D], MDT, tag="xn")
                nc.sync.dma_start(out=xn, in_=x[128 * s:128 * (s + 1), :])
                for dh in range(2):
                    ps = psA.tile([128, QG], MDT, tag="psA")
                    for dj in range(4):
                        d = 4 * dh + dj
                        nc.tensor.transpose(
                            ps[:, 128 * dj:128 * (dj + 1)],
                            xn[:, 128 * d:128 * (d + 1)], ident,
                        )
                    cp = nc.scalar.copy if (si + dh) % 2 == 0 \
                        else nc.vector.tensor_copy
                    cp(xTg[:, 4 * dh:4 * (dh + 1), 128 * si:128 * (si + 1)],
                       ps.rearrange("p (c q) -> p c q", q=128))

            if g == 0:
                # weights + biases (emitted after the first x tiles so x
                # wins the DMA queues; only QKV matmuls gate on these)
                for nm, wap in (("q", wq), ("k", wk), ("v", wv)):
                    t = consts.tile([128, NDCH, 2 * 128], MDT, tag=f"w{nm}",
                                    name=f"w{nm}")
                    nc.sync.dma_start(
                        out=t, in_=wap.rearrange("(c p) m -> p c m", p=128))
                    w_sb[nm] = t
                wo_sb = consts.tile([128, NPAIR, D], MDT, tag="wo")
                nc.sync.dma_start(
                    out=wo_sb, in_=wo.rearrange("(pair p) m -> p pair m", p=128))
                for nm, bap in (("q", bq), ("k", bk), ("v", bv)):
                    t = consts.tile([128, NPAIR], F32, tag=f"b{nm}",
                                    name=f"b{nm}")
                    nc.sync.dma_start(
                        out=t, in_=bap.rearrange("(pair p) -> p pair", p=128))
                    b_sb[nm] = t

            for pair in range(NPAIR):
                for nm, dstT in (("q", qT), ("k", kT), ("v", vT)):
                    psq = psQ.tile([128, QG], F32, tag="psQ")
                    for d in range(NDCH):
                        nc.tensor.matmul(
                            psq,
                            fr(w_sb[nm][:, d, 128 * pair:128 * (pair + 1)]),
                            fr(xTg[:, d, :]),
                            start=(d == 0), stop=(d == NDCH - 1),
                        )
                    nc.vector.tensor_scalar_add(
                        dstT[:, pair, QG * g:QG * (g + 1)], psq,
                        b_sb[nm][:, pair:pair + 1],
                    )

                if g == 0:
                    nc.vector.tensor_copy(
                        vaug[pair][:, :, :, HD:HD + 1],
                        onesf.broadcast_to((128, NKT, 2, 1)),
                    )
                # v natural for this query group's key tiles
                ps = psA.tile([128, QG], MDT, tag="psA")
                for j in range(4):
                    kt = 4 * g + j
                    nc.tensor.transpose(
                        ps[:, 128 * j:128 * (j + 1)],
                        vT[:, pair, 128 * kt:128 * (kt + 1)], ident,
                    )
                cp = nc.scalar.copy if (g + pair) % 2 == 0 \
                    else nc.vector.tensor_copy
                cp(
                    vaug[pair][:, 4 * g:4 * (g + 1), :, 0:HD],
                    ps.rearrange("p (j h d) -> p j h d", j=4, h=2),
                )


    # ------- phase B+C: attention (kg-pipelined) with interleaved out-proj -----
    with ExitStack() as phb:
        psS = phb.enter_context(tc.tile_pool(name="psS", bufs=2, space="PSUM"))
        psAV = phb.enter_context(tc.tile_pool(name="psAV", bufs=2, space="PSUM"))
        psO = phb.enter_context(tc.tile_pool(name="psO", bufs=2, space="PSUM"))
        pt_pool = phb.enter_context(tc.tile_pool(name="pt", bufs=12))
        rb_pool = phb.enter_context(tc.tile_pool(name="rb", bufs=3))
        ob_pool = phb.enter_context(tc.tile_pool(name="ob", bufs=5))

        first_kt = {g: _group_kts(g)[0] for g in range(NQG)}
        last_kt = {g: _group_kts(g)[-1] for g in range(NQG)}

        def outproj_group(g):
            for qt in range(4 * g, 4 * (g + 1)):
                for dh in range(2):
                    pso = psO.tile([128, QG], F32, tag="psO", name="pso")
                    for pair in range(NPAIR):
                        nc.tensor.matmul(
                            pso,
                            U2[:, pair, 128 * qt:128 * (qt + 1)],
                            wo_sb[:, pair, QG * dh:QG * (dh + 1)],
                            start=(pair == 0), stop=(pair == NPAIR - 1),
                        )
                    ob = ob_pool.tile([128, QG], F32, tag="ob")
                    nc.vector.tensor_copy(ob, pso)
                    nc.sync.dma_start(
                        out=out[128 * qt:128 * (qt + 1), QG * dh:QG * (dh + 1)],
                        in_=ob,
                    )

        def emit_av_group(pair, g, pts):
            """AV + normalization for query group g, both heads; pts maps
            (h, kt) -> (pt_tile, j)."""
            for h in range(2):
                psav = psAV.tile([65, QG], F32, tag="psAV", name="psav")
                nc.tensor.matmul(psav, zcol, zrow, start=True, stop=False)
                for kt in _group_kts(g):
                    pt, j = pts[(h, kt)]
                    lo, hi, plo = next(
                        (s[1], s[2], s[3]) for s in _av_slices(kt) if s[0] == g)
                    nc.tensor.matmul(
                        psav[:, plo:plo + (hi - lo)],
                        vaug[pair][:, kt, h, :],
                        pt[:, j, lo:hi],
                        start=False, stop=(kt == last_kt[g]),
                    )
                rt0 = rb_pool.tile([1, QG], F32, tag="rt0")
                nc.scalar.copy(rt0, psav[64:65, :])
                rtmp = rb_pool.tile([1, QG], F32, tag="rtmp")
                nc.vector.reciprocal_approx_fast(out=rtmp, in_=rt0)
                rbt = rb_pool.tile([64, QG], F32, tag="rb")
                nc.gpsimd.partition_broadcast(rbt, rtmp)
                nc.vector.tensor_mul(
                    U2[64 * h:64 * (h + 1), pair, QG * g:QG * (g + 1)],
                    psav[0:64, :], rbt,
                )
            if pair == NPAIR - 1:
                outproj_group(g)

        for pair in range(NPAIR):
            pts = {}
            for kg in range(NKT // 2):
                kts = [2 * kg, 2 * kg + 1]
                pss = [psS.tile([128, 2, QG], F32, tag="psS", name="pss")
                       for _ in range(2)]
                for j, kt in enumerate(kts):
                    q0 = 128 * kt
                    span = min(SPAN, N - q0)
                    for h in range(2):
                        hb = 64 * h
                        nc.tensor.matmul(
                            pss[h][:, j, 0:span],
                            kT[hb:hb + 64, pair, q0:q0 + 128],
                            qT[hb:hb + 64, pair, q0:q0 + span],
                            start=True, stop=True,
                        )
                for h in range(2):
                    pt = pt_pool.tile([128, 2, SPAN], MDT, tag="pt")
                    if kg < NKT // 2 - 1:
                        nc.scalar.activation(
                            pt[:, :, :], pss[h][:, :, 0:SPAN], Exp, scale=SCALE
                        )
                        nc.vector.tensor_mul(
                            pt[:, :, 0:128], pt[:, :, 0:128],
                            mask2[:, :, 0:128],
                        )
                        nc.vector.tensor_mul(
                            pt[:, :, WIN:SPAN], pt[:, :, WIN:SPAN],
                            mask2[:, :, WIN:SPAN],
                        )
                    else:
                        for j, kt in enumerate(kts):
                            span = min(SPAN, N - 128 * kt)
                            nc.scalar.activation(
                                pt[:, j, 0:span], pss[h][:, j, 0:span], Exp,
                                scale=SCALE,
                            )
                            nc.vector.tensor_mul(
                                pt[:, j, 0:128], pt[:, j, 0:128], mask[:, 0:128]
                            )
                            if span > WIN:
                                nc.vector.tensor_mul(
                                    pt[:, j, WIN:span], pt[:, j, WIN:span],
                                    mask[:, WIN:span],
                                )
                    for j, kt in enumerate(kts):
                        pts[(h, kt)] = (pt, j)
                if kg % 2 == 1:
                    emit_av_group(pair, (kg - 1) // 2, pts)


def build(mm_dt=MM_DT):
    nc = bacc.Bacc("TRN2", target_bir_lowering=False, debug=False)
    x = nc.dram_tensor("x", [N, D], mm_dt, kind="ExternalInput").ap()
    wq = nc.dram_tensor("wq", [D, 256], mm_dt, kind="ExternalInput").ap()
    wk = nc.dram_tensor("wk", [D, 256], mm_dt, kind="ExternalInput").ap()
    wv = nc.dram_tensor("wv", [D, 256], mm_dt, kind="ExternalInput").ap()
    wo = nc.dram_tensor("wo", [256, D], mm_dt, kind="ExternalInput").ap()
    bq = nc.dram_tensor("bq", [256], F32, kind="ExternalInput").ap()
    bk = nc.dram_tensor("bk", [256], F32, kind="ExternalInput").ap()
    bv = nc.dram_tensor("bv", [256], F32, kind="ExternalInput").ap()
    out = nc.dram_tensor("out", [N, D], F32, kind="ExternalOutput").ap()
    with tile.TileContext(nc) as tc, ExitStack() as ctx:
        _emit(ctx, tc, (x, wq, wk, wv, wo, bq, bk, bv, out), mm_dt)
    nc.compile()
    return nc


def shard_inputs(x, Wq, bq, Wk, bk, Wv, bv, Wo, bo):
    """Full inputs -> list of 8 per-core input maps."""
    mdt = mybir.dt.np(MM_DT)
    in_maps = []
    for c in range(8):
        b, hg = c // 4, c % 4
        cs = slice(256 * hg, 256 * (hg + 1))
        in_maps.append({
            "x": np.ascontiguousarray(x[b]).astype(mdt),
            "wq": np.ascontiguousarray(Wq[:, cs]).astype(mdt),
            "wk": np.ascontiguousarray(Wk[:, cs]).astype(mdt),
            "wv": np.ascontiguousarray(Wv[:, cs]).astype(mdt),
            "wo": np.ascontiguousarray(Wo[cs, :]).astype(mdt),
            "bq": np.ascontiguousarray(bq[cs]),
            "bk": np.ascontiguousarray(bk[cs]),
            "bv": np.ascontiguousarray(bv[cs]),
        })
    return in_maps


def assemble(results, bo):
    """8 per-core partial outputs -> full [2, N, D] output."""
    outs = [np.asarray(r["out"], dtype=np.float32) for r in results]
    full = np.empty((2, N, D), dtype=np.float32)
    for b in range(2):
        full[b] = outs[4 * b] + outs[4 * b + 1] + outs[4 * b + 2] + outs[4 * b + 3]
        full[b] += bo[None, :]
    return full


_NC_CACHE = {}


def _get_nc():
    key = _MM_DT_NAME
    if key not in _NC_CACHE:
        _NC_CACHE[key] = build()
    return _NC_CACHE[key]


def kernel(x, Wq, bq, Wk, bk, Wv, bv, Wo, bo, _trace=False):
    x, Wq, bq, Wk, bk, Wv, bv, Wo, bo = (
        np.asarray(a, dtype=np.float32)
        for a in (x, Wq, bq, Wk, bk, Wv, bv, Wo, bo)
    )
    nc = _get_nc()
    in_maps = shard_inputs(x, Wq, bq, Wk, bk, Wv, bv, Wo, bo)
    res = run_bass_kernel_spmd(nc, in_maps, core_ids=list(range(8)), trace=_trace)
    full = assemble(res.results, bo)
    if _trace:
        kernel.last_result = res
    return full



# revision 12
# speedup vs baseline: 1.2685x; 1.1400x over previous
"""Trainium2 Bass kernel for nn_LocalSwarmAggregator (sliding-window causal MHA).

Reference computation (fp32):
    q,k,v = x@Wq+bq, x@Wk+bk, x@Wv+bv          # [B,N,D] -> per-head [B,H,N,64]
    logits = q k^T / 8 + band_mask              # causal + 256-window
    out = softmax(logits) v                     # [B,H,N,64]
    y = concat_heads(out) @ Wo + bo             # [B,N,D]

Sharding over 8 cores: core c handles batch c//4 and heads 4*(c%4)..4*(c%4)+3
(tensor-parallel on the head dim of Wq/Wk/Wv and the row dim of Wo).  Each
core computes a partial y (bf16) for its batch; the host sums the 4 partials
per batch in fp32 and adds bo.  No cross-device communication.

v2 layout (all bf16 storage/matmul, fp32 PSUM):
  - x^T is pre-transposed on the HOST; the kernel DMAs xT [D,N] directly, so
    no on-chip x transposes are needed.  QKV projections contract over D with
    the weight chunk stationary and xT moving (512-query groups).
  - v^T is transposed back to v-natural on the PE (identity matmul) and
    augmented with a ones column so attention-weight row sums fall out of the
    AV matmul for free.
  - S^T tiles [128 keys, span<=384 queries] per (kt, pair): for key tile kt
    the only queries attending are 128*kt .. 128*kt+383 and the valid band is
    r <= c <= r+256 -> one constant 0/1 mask.  P^T = exp(S^T/8)*mask (logits
    are O(6) so exp needs no row-max).
  - AV accumulates out^T_aug[65, q] in PSUM per 512-query group; each psum
    column's first contribution uses start=True (fresh/accum split at the
    WIN boundary), so no zero-init matmul is needed.  Row 64 = denominators.
  - The AV psum is copied raw to SBUF immediately (frees the PSUM bank), then
    reciprocal + gpsimd partition_broadcast + multiply produce normalized U2
    off the critical path.  The output projection for group g is interleaved
    into group g+1's S loop to hide the normalization latency.
"""

import os
from contextlib import ExitStack

import numpy as np

import concourse.bass as bass
import concourse.mybir as mybir
import concourse.tile as tile
from concourse import bacc
from concourse.bass_utils import run_bass_kernel_spmd
from concourse.masks import make_identity

F32 = mybir.dt.float32
N = 2048
D = 1024
HD = 64
WIN = 256
NPAIR = 2  # head pairs per core (4 heads)
NCH = D // 128  # 8 contraction chunks
NKT = N // 128  # 16 key tiles
SPAN = 384  # max query span per S^T key tile
QG = 512  # query group size
NQG = N // QG  # 4
SCALE = 1.0 / np.sqrt(HD)

_MM_DT_NAME = os.environ.get("BASS_MM_DT", "bfloat16")
MM_DT = getattr(mybir.dt, _MM_DT_NAME)

Exp = mybir.ActivationFunctionType.Exp
IS_GE = mybir.AluOpType.is_ge


def _span(kt):
    return min(SPAN, N - 128 * kt)


def _av_slices(kt):
    """For key tile kt return [(g, lo, hi, plo)]: query-group g consumes
    P^T[kt][:, lo:hi] into psum columns plo:plo+(hi-lo)."""
    span = _span(kt)
    out = []
    for g in range(NQG):
        lo = max(0, QG * g - 128 * kt)
        hi = min(span, QG * g + QG - 128 * kt)
        if lo < hi:
            out.append((g, lo, hi, 128 * kt + lo - QG * g))
    return out


def _group_kts(g):
    """Key tiles contributing to query group g (ordered)."""
    return [kt for kt in range(NKT) if any(s[0] == g for s in _av_slices(kt))]


def _av_pieces(g):
    """[(kt, lo, hi, plo, fresh)]: matmul pieces for group g's AV psum.
    A column's first contribution (fresh) overwrites (start=True); later
    kts accumulate.  Fresh region of kt is cols [WIN, span) (kt==0: all)."""
    pieces = []
    for kt in _group_kts(g):
        (lo, hi, plo) = next((s[1], s[2], s[3]) for s in _av_slices(kt) if s[0] == g)
        if kt == 0:
            pieces.append((kt, lo, hi, plo, True))
            continue
        if lo < WIN and lo < hi:
            pieces.append((kt, lo, min(hi, WIN), plo, False))
        if hi > WIN:
            w0 = max(lo, WIN)
            pieces.append((kt, w0, hi, plo + (w0 - lo), True))
    return pieces


def _emit(ctx: ExitStack, tc: tile.TileContext, aps, mm_dt, debug_taps=None):
    nc = tc.nc
    xT, wq, wk, wv, wo, bq, bk, bv, out = aps
    MDT = mm_dt

    consts = ctx.enter_context(tc.tile_pool(name="consts", bufs=1))
    persist = ctx.enter_context(tc.tile_pool(name="persist", bufs=1))

    ident_f = consts.tile([128, 128], F32, tag="ident_f")
    make_identity(nc, ident_f)
    ident = consts.tile([128, 128], MDT, tag="ident")
    nc.vector.tensor_copy(ident, ident_f)

    # 0/1 band mask: valid iff r <= c <= r + WIN (keys on partitions, query
    # offset on free dim).  Only cols [0:128) and [WIN:SPAN) are ever partial.
    mask_f = consts.tile([128, SPAN], F32, tag="mask_f")
    nc.gpsimd.memset(mask_f, 1.0)
    nc.gpsimd.affine_select(
        out=mask_f, in_=mask_f, compare_op=IS_GE, fill=0.0,
        base=0, pattern=[[1, SPAN]], channel_multiplier=-1,
    )  # keep c - r >= 0
    nc.gpsimd.affine_select(
        out=mask_f, in_=mask_f, compare_op=IS_GE, fill=0.0,
        base=WIN, pattern=[[-1, SPAN]], channel_multiplier=1,
    )  # keep r - c + WIN >= 0
    mask2 = consts.tile([128, 2, SPAN], MDT, tag="mask2")
    nc.vector.tensor_copy(mask2[:, 0, :], mask_f)
    nc.vector.tensor_copy(mask2[:, 1, :], mask_f)

    onesf = consts.tile([128, 1], F32, tag="onesf")
    nc.vector.memset(onesf, 1.0)
    zf = consts.tile([1, QG], F32, tag="zf")
    nc.vector.memset(zf, 0.0)
    zcol = consts.tile([1, 65], MDT, tag="zcol")
    nc.vector.tensor_copy(zcol, zf[:, 0:65])
    zrow = consts.tile([1, QG], MDT, tag="zrow")
    nc.vector.tensor_copy(zrow, zf)

    # persistent intermediates
    qT = persist.tile([128, NPAIR, N], MDT, tag="qT")
    kT = persist.tile([128, NPAIR, N], MDT, tag="kT")
    vaug = [persist.tile([128, NKT, 2, HD + 1], MDT, tag=f"vaug{p}",
                         name=f"vaug{p}")
            for p in range(NPAIR)]
    U2 = persist.tile([128, NPAIR, N], MDT, tag="U2")

    # ---- input DMAs (ordered so group-0 work can start ASAP) ----
    xTr = xT.rearrange("(c p) n -> p c n", p=128)
    xT_sb = consts.tile([128, NCH, N], MDT, tag="xT_sb")
    w_sb = {}
    b_sb = {}

    t = consts.tile([128, NCH, 2 * 128], MDT, tag="wq")
    nc.sync.dma_start(out=t, in_=wq.rearrange("(c p) m -> p c m", p=128))
    w_sb["q"] = t
    for nm, bap in (("q", bq), ("k", bk), ("v", bv)):
        t = consts.tile([128, NPAIR], F32, tag=f"b{nm}", name=f"b{nm}")
        nc.sync.dma_start(out=t, in_=bap.rearrange("(pair p) -> p pair", p=128))
        b_sb[nm] = t
    nc.sync.dma_start(out=xT_sb[:, :, 0:QG], in_=xTr[:, :, 0:QG])
    for nm, wap in (("k", wk), ("v", wv)):
        t = consts.tile([128, NCH, 2 * 128], MDT, tag=f"w{nm}", name=f"w{nm}")
        nc.sync.dma_start(out=t, in_=wap.rearrange("(c p) m -> p c m", p=128))
        w_sb[nm] = t
    nc.sync.dma_start(out=xT_sb[:, :, QG:2 * QG], in_=xTr[:, :, QG:2 * QG])
    wo_sb = consts.tile([128, NPAIR, D], MDT, tag="wo")
    nc.sync.dma_start(out=wo_sb, in_=wo.rearrange("(pair p) m -> p pair m", p=128))
    nc.sync.dma_start(out=xT_sb[:, :, 2 * QG:3 * QG], in_=xTr[:, :, 2 * QG:3 * QG])
    nc.sync.dma_start(out=xT_sb[:, :, 3 * QG:4 * QG], in_=xTr[:, :, 3 * QG:4 * QG])

    # ---------------- phase A: QKV projections + v_aug ----------------
    with ExitStack() as pha:
        psQ = pha.enter_context(tc.tile_pool(name="psQ", bufs=4, space="PSUM"))
        psT = pha.enter_context(tc.tile_pool(name="psT", bufs=2, space="PSUM"))
        vt_pool = pha.enter_context(tc.tile_pool(name="vt", bufs=2))

        for pair in range(NPAIR):
            nc.vector.tensor_copy(
                vaug[pair][:, :, :, HD:HD + 1],
                onesf.broadcast_to((128, NKT, 2, 1)),
            )

        for g in range(NQG):
            gsl = slice(QG * g, QG * (g + 1))
            vt = vt_pool.tile([128, NPAIR, QG], MDT, tag="vt")
            for pair in range(NPAIR):
                for nm, dstT in (("q", qT), ("k", kT), ("v", None)):
                    psq = psQ.tile([128, QG], F32, tag="psq")
                    for c in range(NCH):
                        nc.tensor.matmul(
                            psq,
                            w_sb[nm][:, c, 128 * pair:128 * (pair + 1)],
                            xT_sb[:, c, gsl],
                            start=(c == 0), stop=(c == NCH - 1),
                        )
                    dst = vt[:, pair, :] if dstT is None else dstT[:, pair, gsl]
                    nc.vector.tensor_scalar_add(
                        dst, psq, b_sb[nm][:, pair:pair + 1])
            for pair in range(NPAIR):
                pst = psT.tile([128, QG], MDT, tag="pst")
                for j in range(4):
                    nc.tensor.transpose(
                        pst[:, 128 * j:128 * (j + 1)],
                        vt[:, pair, 128 * j:128 * (j + 1)], ident,
                    )
                nc.vector.tensor_copy(
                    vaug[pair][:, 4 * g:4 * (g + 1), :, 0:HD],
                    pst.rearrange("p (j h d) -> p j h d", j=4, h=2),
                )

    # ------- phase B: attention + interleaved out-projection -------
    with ExitStack() as phb:
        psS = phb.enter_context(tc.tile_pool(name="psS", bufs=2, space="PSUM"))
        psAV = phb.enter_context(tc.tile_pool(name="psAV", bufs=2, space="PSUM"))
        psO = phb.enter_context(tc.tile_pool(name="psO", bufs=2, space="PSUM"))
        pt_pool = phb.enter_context(tc.tile_pool(name="pt", bufs=14))
        u2r_pool = phb.enter_context(tc.tile_pool(name="u2r", bufs=3))
        rb_pool = phb.enter_context(tc.tile_pool(name="rb", bufs=3))
        ob_pool = phb.enter_context(tc.tile_pool(name="ob", bufs=4))

        pts = {}

        def s_unit(kt, pair):
            span = _span(kt)
            q0 = 128 * kt
            pss = psS.tile([128, 2, QG], F32, tag="pss")
            for h in range(2):
                hb = 64 * h
                nc.tensor.matmul(
                    pss[:, h, 0:span],
                    kT[hb:hb + 64, pair, q0:q0 + 128],
                    qT[hb:hb + 64, pair, q0:q0 + span],
                    start=True, stop=True,
                )
            pt = pt_pool.tile([128, 2, SPAN], MDT, tag="pt")
            nc.scalar.activation(
                pt[:, :, 0:span], pss[:, :, 0:span], Exp, scale=SCALE)
            nc.vector.tensor_mul(
                pt[:, :, 0:128], pt[:, :, 0:128], mask2[:, :, 0:128])
            if span > WIN:
                nc.vector.tensor_mul(
                    pt[:, :, WIN:span], pt[:, :, WIN:span],
                    mask2[:, :, WIN:span])
            if debug_taps is not None and pair == 0 and kt < 4:
                nc.sync.dma_start(out=debug_taps[0][:, kt, :, :], in_=pt)
            pts[(pair, kt)] = pt

        def av_unit(g, pair, h):
            pieces = _av_pieces(g)
            psav = psAV.tile([65, QG], F32, tag="psav")
            nc.tensor.matmul(psav, zcol, zrow, start=True, stop=False)
            for i, (kt, lo, hi, plo, fresh) in enumerate(pieces):
                nc.tensor.matmul(
                    psav[:, plo:plo + (hi - lo)],
                    vaug[pair][:, kt, h, :],
                    pts[(pair, kt)][:, h, lo:hi],
                    start=False, stop=(i == len(pieces) - 1),
                )
            u2rn = u2r_pool.tile([64, QG], F32, tag="u2rn")
            nc.vector.tensor_copy(u2rn, psav[0:64, :])
            rt0 = rb_pool.tile([1, QG], F32, tag="rt0")
            nc.scalar.copy(rt0, psav[64:65, :])
            rtmp = rb_pool.tile([1, QG], F32, tag="rtmp")
            nc.vector.reciprocal_approx_fast(out=rtmp, in_=rt0)
            rbt = rb_pool.tile([64, QG], F32, tag="rbt")
            nc.gpsimd.partition_broadcast(rbt, rtmp)
            nc.vector.tensor_mul(
                U2[64 * h:64 * (h + 1), pair, QG * g:QG * (g + 1)],
                u2rn, rbt,
            )
            if debug_taps is not None and len(debug_taps) > 2 \
                    and g == 0 and pair == 0 and h == 0:
                nc.sync.dma_start(out=debug_taps[2][0:1, :], in_=rt0)
                nc.sync.dma_start(out=debug_taps[2][1:2, :], in_=rtmp)
                nc.sync.dma_start(out=debug_taps[2][2:3, :], in_=rbt[0:1, :])
                nc.sync.dma_start(out=debug_taps[2][3:4, :], in_=u2rn[0:1, :])

        def o_block(g, qt, dh):
            pso = psO.tile([128, QG], F32, tag="pso")
            for pair in range(NPAIR):
                nc.tensor.matmul(
                    pso,
                    U2[:, pair, 128 * qt:128 * (qt + 1)],
                    wo_sb[:, pair, QG * dh:QG * (dh + 1)],
                    start=(pair == 0), stop=(pair == NPAIR - 1),
                )
            ob = ob_pool.tile([128, QG], MDT, tag="ob")
            nc.vector.tensor_copy(ob, pso)
            nc.sync.dma_start(
                out=out[128 * qt:128 * (qt + 1), QG * dh:QG * (dh + 1)],
                in_=ob,
            )

        for g in range(NQG):
            for i, kt in enumerate(range(4 * g, 4 * (g + 1))):
                s_unit(kt, 0)
                s_unit(kt, 1)
                if g >= 1:
                    gp = g - 1
                    for qt, dh in [(4 * gp + i, 0), (4 * gp + i, 1)]:
                        o_block(gp, qt, dh)
            for pair in range(NPAIR):
                for h in range(2):
                    av_unit(g, pair, h)
        for qt in range(12, 16):
            for dh in range(2):
                o_block(3, qt, dh)
        if debug_taps is not None:
            nc.sync.dma_start(out=debug_taps[1], in_=U2)


def build(mm_dt=MM_DT):
    nc = bacc.Bacc("TRN2", target_bir_lowering=False, debug=False)
    xT = nc.dram_tensor("xT", [D, N], mm_dt, kind="ExternalInput").ap()
    wq = nc.dram_tensor("wq", [D, 256], mm_dt, kind="ExternalInput").ap()
    wk = nc.dram_tensor("wk", [D, 256], mm_dt, kind="ExternalInput").ap()
    wv = nc.dram_tensor("wv", [D, 256], mm_dt, kind="ExternalInput").ap()
    wo = nc.dram_tensor("wo", [256, D], mm_dt, kind="ExternalInput").ap()
    bq = nc.dram_tensor("bq", [256], F32, kind="ExternalInput").ap()
    bk = nc.dram_tensor("bk", [256], F32, kind="ExternalInput").ap()
    bv = nc.dram_tensor("bv", [256], F32, kind="ExternalInput").ap()
    out = nc.dram_tensor("out", [N, D], mm_dt, kind="ExternalOutput").ap()
    with tile.TileContext(nc) as tc, ExitStack() as ctx:
        _emit(ctx, tc, (xT, wq, wk, wv, wo, bq, bk, bv, out), mm_dt)
    nc.compile()
    return nc


def shard_inputs(x, Wq, bq, Wk, bk, Wv, bv, Wo, bo):
    """Full inputs -> list of 8 per-core input maps (host pre-transposes x)."""
    mdt = mybir.dt.np(MM_DT)
    xTb = [np.ascontiguousarray(x[b].T).astype(mdt) for b in range(2)]
    in_maps = []
    for c in range(8):
        b, hg = c // 4, c % 4
        cs = slice(256 * hg, 256 * (hg + 1))
        in_maps.append({
            "xT": xTb[b],
            "wq": np.ascontiguousarray(Wq[:, cs]).astype(mdt),
            "wk": np.ascontiguousarray(Wk[:, cs]).astype(mdt),
            "wv": np.ascontiguousarray(Wv[:, cs]).astype(mdt),
            "wo": np.ascontiguousarray(Wo[cs, :]).astype(mdt),
            "bq": np.ascontiguousarray(bq[cs]),
            "bk": np.ascontiguousarray(bk[cs]),
            "bv": np.ascontiguousarray(bv[cs]),
        })
    return in_maps


def assemble(results, bo):
    """8 per-core partial outputs (bf16) -> full [2, N, D] fp32 output."""
    outs = [np.asarray(r["out"], dtype=np.float32) for r in results]
    full = np.empty((2, N, D), dtype=np.float32)
    for b in range(2):
        full[b] = outs[4 * b] + outs[4 * b + 1] + outs[4 * b + 2] + outs[4 * b + 3]
        full[b] += bo[None, :]
    return full


_NC_CACHE = {}


def _get_nc():
    key = _MM_DT_NAME
    if key not in _NC_CACHE:
        _NC_CACHE[key] = build()
    return _NC_CACHE[key]


def kernel(x, Wq, bq, Wk, bk, Wv, bv, Wo, bo, _trace=False):
    x, Wq, bq, Wk, bk, Wv, bv, Wo, bo = (
        np.asarray(a, dtype=np.float32)
        for a in (x, Wq, bq, Wk, bk, Wv, bv, Wo, bo)
    )
    nc = _get_nc()
    in_maps = shard_inputs(x, Wq, bq, Wk, bk, Wv, bv, Wo, bo)
    res = run_bass_kernel_spmd(nc, in_maps, core_ids=list(range(8)), trace=_trace)
    full = assemble(res.results, bo)
    if _trace:
        kernel.last_result = res
    return full


# revision 14
# speedup vs baseline: 1.2810x; 1.0099x over previous
"""Trainium2 Bass kernel for nn_LocalSwarmAggregator (sliding-window causal MHA).

Reference computation (fp32):
    q,k,v = x@Wq+bq, x@Wk+bk, x@Wv+bv          # [B,N,D] -> per-head [B,H,N,64]
    logits = q k^T / 8 + band_mask              # causal + 256-window
    out = softmax(logits) v                     # [B,H,N,64]
    y = concat_heads(out) @ Wo + bo             # [B,N,D]

Sharding over 8 cores: core c handles batch c//4 and heads 4*(c%4)..4*(c%4)+3
(tensor-parallel on the head dim of Wq/Wk/Wv and the row dim of Wo).  Each
core computes a partial y (bf16) for its batch; the host sums the 4 partials
per batch in fp32 and adds bo.  No cross-device communication.

v2 layout (all bf16 storage/matmul, fp32 PSUM):
  - x^T is pre-transposed on the HOST; the kernel DMAs xT [D,N] directly, so
    no on-chip x transposes are needed.  QKV projections contract over D with
    the weight chunk stationary and xT moving (512-query groups).
  - v^T is transposed back to v-natural on the PE (identity matmul) and
    augmented with a ones column so attention-weight row sums fall out of the
    AV matmul for free.
  - S^T tiles [128 keys, span<=384 queries] per (kt, pair): for key tile kt
    the only queries attending are 128*kt .. 128*kt+383 and the valid band is
    r <= c <= r+256 -> one constant 0/1 mask.  P^T = exp(S^T/8)*mask (logits
    are O(6) so exp needs no row-max).
  - AV accumulates out^T_aug[65, q] in PSUM per 512-query group; each psum
    column's first contribution uses start=True (fresh/accum split at the
    WIN boundary), so no zero-init matmul is needed.  Row 64 = denominators.
  - The AV psum is copied raw to SBUF immediately (frees the PSUM bank), then
    reciprocal + gpsimd partition_broadcast + multiply produce normalized U2
    off the critical path.  The output projection for group g is interleaved
    into group g+1's S loop to hide the normalization latency.
"""

import os
from contextlib import ExitStack

import numpy as np

import concourse.bass as bass
import concourse.mybir as mybir
import concourse.tile as tile
from concourse import bacc
from concourse.bass_utils import run_bass_kernel_spmd
from concourse.masks import make_identity

F32 = mybir.dt.float32
N = 2048
D = 1024
HD = 64
WIN = 256
NPAIR = 2  # head pairs per core (4 heads)
NCH = D // 128  # 8 contraction chunks
NKT = N // 128  # 16 key tiles
SPAN = 384  # max query span per S^T key tile
QG = 512  # query group size
NQG = N // QG  # 4
SCALE = 1.0 / np.sqrt(HD)

_MM_DT_NAME = os.environ.get("BASS_MM_DT", "bfloat16")
MM_DT = getattr(mybir.dt, _MM_DT_NAME)

Exp = mybir.ActivationFunctionType.Exp
IS_GE = mybir.AluOpType.is_ge


def _span(kt):
    return min(SPAN, N - 128 * kt)


def _av_slices(kt):
    """For key tile kt return [(g, lo, hi, plo)]: query-group g consumes
    P^T[kt][:, lo:hi] into psum columns plo:plo+(hi-lo)."""
    span = _span(kt)
    out = []
    for g in range(NQG):
        lo = max(0, QG * g - 128 * kt)
        hi = min(span, QG * g + QG - 128 * kt)
        if lo < hi:
            out.append((g, lo, hi, 128 * kt + lo - QG * g))
    return out


def _group_kts(g):
    """Key tiles contributing to query group g (ordered)."""
    return [kt for kt in range(NKT) if any(s[0] == g for s in _av_slices(kt))]


def _av_pieces(g):
    """[(kt, lo, hi, plo, fresh)]: matmul pieces for group g's AV psum.
    A column's first contribution (fresh) overwrites (start=True); later
    kts accumulate.  Fresh region of kt is cols [WIN, span) (kt==0: all)."""
    pieces = []
    for kt in _group_kts(g):
        (lo, hi, plo) = next((s[1], s[2], s[3]) for s in _av_slices(kt) if s[0] == g)
        if kt == 0:
            pieces.append((kt, lo, hi, plo, True))
            continue
        if lo < WIN and lo < hi:
            pieces.append((kt, lo, min(hi, WIN), plo, False))
        if hi > WIN:
            w0 = max(lo, WIN)
            pieces.append((kt, w0, hi, plo + (w0 - lo), True))
    return pieces


def _emit(ctx: ExitStack, tc: tile.TileContext, aps, mm_dt, debug_taps=None):
    nc = tc.nc
    xT, wq, wk, wv, wo, bq, bk, bv, out = aps
    MDT = mm_dt

    consts = ctx.enter_context(tc.tile_pool(name="consts", bufs=1))
    persist = ctx.enter_context(tc.tile_pool(name="persist", bufs=1))

    ident_f = consts.tile([128, 128], F32, tag="ident_f")
    make_identity(nc, ident_f)
    ident = consts.tile([128, 128], MDT, tag="ident")
    nc.vector.tensor_copy(ident, ident_f)

    # 0/1 band mask: valid iff r <= c <= r + WIN (keys on partitions, query
    # offset on free dim).  Only cols [0:128) and [WIN:SPAN) are ever partial.
    mask_f = consts.tile([128, SPAN], F32, tag="mask_f")
    nc.gpsimd.memset(mask_f, 1.0)
    nc.gpsimd.affine_select(
        out=mask_f, in_=mask_f, compare_op=IS_GE, fill=0.0,
        base=0, pattern=[[1, SPAN]], channel_multiplier=-1,
    )  # keep c - r >= 0
    nc.gpsimd.affine_select(
        out=mask_f, in_=mask_f, compare_op=IS_GE, fill=0.0,
        base=WIN, pattern=[[-1, SPAN]], channel_multiplier=1,
    )  # keep r - c + WIN >= 0
    mask2 = consts.tile([128, 2, SPAN], MDT, tag="mask2")
    nc.vector.tensor_copy(mask2[:, 0, :], mask_f)
    nc.vector.tensor_copy(mask2[:, 1, :], mask_f)

    onesf = consts.tile([128, 1], F32, tag="onesf")
    nc.vector.memset(onesf, 1.0)
    zf = consts.tile([1, QG], F32, tag="zf")
    nc.vector.memset(zf, 0.0)
    zcol = consts.tile([1, 65], MDT, tag="zcol")
    nc.vector.tensor_copy(zcol, zf[:, 0:65])
    zrow = consts.tile([1, QG], MDT, tag="zrow")
    nc.vector.tensor_copy(zrow, zf)

    # persistent intermediates
    qT = persist.tile([128, NPAIR, N], MDT, tag="qT")
    kT = persist.tile([128, NPAIR, N], MDT, tag="kT")
    vaug = [persist.tile([128, NKT, 2, HD + 1], MDT, tag=f"vaug{p}",
                         name=f"vaug{p}")
            for p in range(NPAIR)]
    U2 = persist.tile([128, NPAIR, N], MDT, tag="U2")

    # ---- input DMAs (ordered so group-0 work can start ASAP) ----
    xTr = xT.rearrange("(c p) n -> p c n", p=128)
    xT_sb = consts.tile([128, NCH, N], MDT, tag="xT_sb")
    w_sb = {}
    b_sb = {}

    t = consts.tile([128, NCH, 2 * 128], MDT, tag="wq")
    nc.sync.dma_start(out=t, in_=wq.rearrange("(c p) m -> p c m", p=128))
    w_sb["q"] = t
    for nm, bap in (("q", bq), ("k", bk), ("v", bv)):
        t = consts.tile([128, NPAIR], F32, tag=f"b{nm}", name=f"b{nm}")
        nc.sync.dma_start(out=t, in_=bap.rearrange("(pair p) -> p pair", p=128))
        b_sb[nm] = t
    nc.sync.dma_start(out=xT_sb[:, :, 0:QG], in_=xTr[:, :, 0:QG])
    for nm, wap in (("k", wk), ("v", wv)):
        t = consts.tile([128, NCH, 2 * 128], MDT, tag=f"w{nm}", name=f"w{nm}")
        nc.sync.dma_start(out=t, in_=wap.rearrange("(c p) m -> p c m", p=128))
        w_sb[nm] = t
    nc.sync.dma_start(out=xT_sb[:, :, QG:2 * QG], in_=xTr[:, :, QG:2 * QG])
    wo_sb = consts.tile([128, NPAIR, D], MDT, tag="wo")
    nc.sync.dma_start(out=wo_sb, in_=wo.rearrange("(pair p) m -> p pair m", p=128))
    nc.sync.dma_start(out=xT_sb[:, :, 2 * QG:3 * QG], in_=xTr[:, :, 2 * QG:3 * QG])
    nc.sync.dma_start(out=xT_sb[:, :, 3 * QG:4 * QG], in_=xTr[:, :, 3 * QG:4 * QG])

    # ---------------- phase A: QKV projections + v_aug ----------------
    with ExitStack() as pha:
        psQ = pha.enter_context(tc.tile_pool(name="psQ", bufs=4, space="PSUM"))
        psT = pha.enter_context(tc.tile_pool(name="psT", bufs=2, space="PSUM"))
        vt_pool = pha.enter_context(tc.tile_pool(name="vt", bufs=2))

        for pair in range(NPAIR):
            nc.vector.tensor_copy(
                vaug[pair][:, :, :, HD:HD + 1],
                onesf.broadcast_to((128, NKT, 2, 1)),
            )

        for g in range(NQG):
            gsl = slice(QG * g, QG * (g + 1))
            vt = vt_pool.tile([128, NPAIR, QG], MDT, tag="vt")
            for pair in range(NPAIR):
                for nm, dstT in (("q", qT), ("k", kT), ("v", None)):
                    psq = psQ.tile([128, QG], F32, tag="psq")
                    for c in range(NCH):
                        nc.tensor.matmul(
                            psq,
                            w_sb[nm][:, c, 128 * pair:128 * (pair + 1)],
                            xT_sb[:, c, gsl],
                            start=(c == 0), stop=(c == NCH - 1),
                        )
                    dst = vt[:, pair, :] if dstT is None else dstT[:, pair, gsl]
                    nc.vector.tensor_scalar_add(
                        dst, psq, b_sb[nm][:, pair:pair + 1])
            for pair in range(NPAIR):
                pst = psT.tile([128, QG], MDT, tag="pst")
                for j in range(4):
                    nc.tensor.transpose(
                        pst[:, 128 * j:128 * (j + 1)],
                        vt[:, pair, 128 * j:128 * (j + 1)], ident,
                    )
                nc.vector.tensor_copy(
                    vaug[pair][:, 4 * g:4 * (g + 1), :, 0:HD],
                    pst.rearrange("p (j h d) -> p j h d", j=4, h=2),
                )

    # ------- phase B: attention + interleaved out-projection -------
    with ExitStack() as phb:
        psS = phb.enter_context(tc.tile_pool(name="psS", bufs=2, space="PSUM"))
        psAV = phb.enter_context(tc.tile_pool(name="psAV", bufs=2, space="PSUM"))
        psO = phb.enter_context(tc.tile_pool(name="psO", bufs=2, space="PSUM"))
        pt_pool = phb.enter_context(tc.tile_pool(name="pt", bufs=14))
        u2r_pool = phb.enter_context(tc.tile_pool(name="u2r", bufs=3))
        rb_pool = phb.enter_context(tc.tile_pool(name="rb", bufs=3))
        ob_pool = phb.enter_context(tc.tile_pool(name="ob", bufs=4))

        pts = {}

        def s_unit(kt, pair):
            span = _span(kt)
            q0 = 128 * kt
            pss = psS.tile([128, 2, QG], F32, tag="pss")
            for h in range(2):
                hb = 64 * h
                nc.tensor.matmul(
                    pss[:, h, 0:span],
                    kT[hb:hb + 64, pair, q0:q0 + 128],
                    qT[hb:hb + 64, pair, q0:q0 + span],
                    start=True, stop=True,
                )
            pt = pt_pool.tile([128, 2, SPAN], MDT, tag="pt")
            nc.scalar.activation(
                pt[:, :, 0:span], pss[:, :, 0:span], Exp, scale=SCALE)
            nc.vector.tensor_mul(
                pt[:, :, 0:128], pt[:, :, 0:128], mask2[:, :, 0:128])
            if span > WIN:
                nc.vector.tensor_mul(
                    pt[:, :, WIN:span], pt[:, :, WIN:span],
                    mask2[:, :, WIN:span])
            if debug_taps is not None and pair == 0 and kt < 4:
                nc.sync.dma_start(out=debug_taps[0][:, kt, :, :], in_=pt)
            pts[(pair, kt)] = pt

        def av_unit(g, pair, h):
            pieces = _av_pieces(g)
            psav = psAV.tile([65, QG], F32, tag="psav")
            nc.tensor.matmul(psav, zcol, zrow, start=True, stop=False)
            for i, (kt, lo, hi, plo, fresh) in enumerate(pieces):
                nc.tensor.matmul(
                    psav[:, plo:plo + (hi - lo)],
                    vaug[pair][:, kt, h, :],
                    pts[(pair, kt)][:, h, lo:hi],
                    start=False, stop=(i == len(pieces) - 1),
                )
            u2rn = u2r_pool.tile([64, QG], F32, tag="u2rn")
            nc.vector.tensor_copy(u2rn, psav[0:64, :])
            rt0 = rb_pool.tile([1, QG], F32, tag="rt0")
            nc.scalar.copy(rt0, psav[64:65, :])
            rtmp = rb_pool.tile([1, QG], F32, tag="rtmp")
            nc.vector.reciprocal_approx_fast(out=rtmp, in_=rt0)
            rbt = rb_pool.tile([64, QG], F32, tag="rbt")
            nc.gpsimd.partition_broadcast(rbt, rtmp)
            nc.vector.tensor_mul(
                U2[64 * h:64 * (h + 1), pair, QG * g:QG * (g + 1)],
                u2rn, rbt,
            )
            if debug_taps is not None and len(debug_taps) > 2 \
                    and g == 0 and pair == 0 and h == 0:
                nc.sync.dma_start(out=debug_taps[2][0:1, :], in_=rt0)
                nc.sync.dma_start(out=debug_taps[2][1:2, :], in_=rtmp)
                nc.sync.dma_start(out=debug_taps[2][2:3, :], in_=rbt[0:1, :])
                nc.sync.dma_start(out=debug_taps[2][3:4, :], in_=u2rn[0:1, :])

        def o_block(g, qt, dh):
            pso = psO.tile([128, QG], F32, tag="pso")
            for pair in range(NPAIR):
                nc.tensor.matmul(
                    pso,
                    U2[:, pair, 128 * qt:128 * (qt + 1)],
                    wo_sb[:, pair, QG * dh:QG * (dh + 1)],
                    start=(pair == 0), stop=(pair == NPAIR - 1),
                )
            ob = ob_pool.tile([128, QG], MDT, tag="ob")
            nc.vector.tensor_copy(ob, pso)
            nc.sync.dma_start(
                out=out[128 * qt:128 * (qt + 1), QG * dh:QG * (dh + 1)],
                in_=ob,
            )

        for g in range(NQG):
            for i, kt in enumerate(range(4 * g, 4 * (g + 1))):
                s_unit(kt, 0)
                s_unit(kt, 1)
                if g >= 1:
                    gp = g - 1
                    for qt, dh in [(4 * gp + i, 0), (4 * gp + i, 1)]:
                        o_block(gp, qt, dh)
            for pair in range(NPAIR):
                for h in range(2):
                    av_unit(g, pair, h)
        for qt in range(12, 16):
            for dh in range(2):
                o_block(3, qt, dh)
        if debug_taps is not None:
            nc.sync.dma_start(out=debug_taps[1], in_=U2)


def build(mm_dt=MM_DT):
    nc = bacc.Bacc("TRN2", target_bir_lowering=False, debug=False)
    xT = nc.dram_tensor("xT", [D, N], mm_dt, kind="ExternalInput").ap()
    wq = nc.dram_tensor("wq", [D, 256], mm_dt, kind="ExternalInput").ap()
    wk = nc.dram_tensor("wk", [D, 256], mm_dt, kind="ExternalInput").ap()
    wv = nc.dram_tensor("wv", [D, 256], mm_dt, kind="ExternalInput").ap()
    wo = nc.dram_tensor("wo", [256, D], mm_dt, kind="ExternalInput").ap()
    bq = nc.dram_tensor("bq", [256], F32, kind="ExternalInput").ap()
    bk = nc.dram_tensor("bk", [256], F32, kind="ExternalInput").ap()
    bv = nc.dram_tensor("bv", [256], F32, kind="ExternalInput").ap()
    out = nc.dram_tensor("out", [N, D], mm_dt, kind="ExternalOutput").ap()
    with tile.TileContext(nc) as tc, ExitStack() as ctx:
        _emit(ctx, tc, (xT, wq, wk, wv, wo, bq, bk, bv, out), mm_dt)
    nc.compile()
    return nc


def shard_inputs(x, Wq, bq, Wk, bk, Wv, bv, Wo, bo):
    """Full inputs -> list of 8 per-core input maps (host pre-transposes x)."""
    mdt = mybir.dt.np(MM_DT)
    xTb = [np.ascontiguousarray(x[b].T).astype(mdt) for b in range(2)]
    in_maps = []
    for c in range(8):
        b, hg = c // 4, c % 4
        cs = slice(256 * hg, 256 * (hg + 1))
        in_maps.append({
            "xT": xTb[b],
            "wq": np.ascontiguousarray(Wq[:, cs]).astype(mdt),
            "wk": np.ascontiguousarray(Wk[:, cs]).astype(mdt),
            "wv": np.ascontiguousarray(Wv[:, cs]).astype(mdt),
            "wo": np.ascontiguousarray(Wo[cs, :]).astype(mdt),
            "bq": np.ascontiguousarray(bq[cs]),
            "bk": np.ascontiguousarray(bk[cs]),
            "bv": np.ascontiguousarray(bv[cs]),
        })
    return in_maps


def assemble(results, bo):
    """8 per-core partial outputs (bf16) -> full [2, N, D] fp32 output."""
    outs = [np.asarray(r["out"], dtype=np.float32) for r in results]
    full = np.empty((2, N, D), dtype=np.float32)
    for b in range(2):
        full[b] = outs[4 * b] + outs[4 * b + 1] + outs[4 * b + 2] + outs[4 * b + 3]
        full[b] += bo[None, :]
    return full


_NC_CACHE = {}


def _get_nc():
    key = _MM_DT_NAME
    if key not in _NC_CACHE:
        _NC_CACHE[key] = build()
    return _NC_CACHE[key]


def kernel(x, Wq, bq, Wk, bk, Wv, bv, Wo, bo, _trace=False):
    x, Wq, bq, Wk, bk, Wv, bv, Wo, bo = (
        np.asarray(a, dtype=np.float32)
        for a in (x, Wq, bq, Wk, bk, Wv, bv, Wo, bo)
    )
    nc = _get_nc()
    in_maps = shard_inputs(x, Wq, bq, Wk, bk, Wv, bv, Wo, bo)
    res = run_bass_kernel_spmd(nc, in_maps, core_ids=list(range(8)), trace=_trace)
    full = assemble(res.results, bo)
    if _trace:
        kernel.last_result = res
    return full


# revision 16
# speedup vs baseline: 1.3401x; 1.0462x over previous
"""Trainium2 Bass kernel for nn_LocalSwarmAggregator (sliding-window causal MHA).

Reference computation (fp32):
    q,k,v = x@Wq+bq, x@Wk+bk, x@Wv+bv          # [B,N,D] -> per-head [B,H,N,64]
    logits = q k^T / 8 + band_mask              # causal + 256-window
    out = softmax(logits) v                     # [B,H,N,64]
    y = concat_heads(out) @ Wo + bo             # [B,N,D]

Sharding over 8 cores: core c handles batch c//4 and heads 4*(c%4)..4*(c%4)+3
(tensor-parallel on the head dim of Wq/Wk/Wv and the row dim of Wo).  Each
core computes a partial y (bf16) for its batch; the host sums the 4 partials
per batch in fp32 and adds bo.  No cross-device communication.

v3: single software-pipelined loop over 512-query groups, all bf16 storage
and matmuls (fp32 PSUM).  The per-group steady state interleaves, on the PE:
QKV chains for group g (x^T is host-pre-transposed, so no x transposes),
S^T = K q^T for key tiles 4g-2..4g+1, AV for group g-1, and the output
projection for group g-2.  This keeps the scalar engine's exp stream (the
2nd-busiest engine) fully hidden under PE work, and keeps the PE dense so it
stays at its max p-state.  Host-side arrays are pre-arranged so every DMA is
contiguous per partition line (descriptor issue time, not bandwidth, was the
startup bottleneck).

PSUM (8 banks): acc ring 2 (QKV chains + out-proj blocks share one tag),
S^T 2x[128,2,512] = 4, AV [65,512] = 1, v-transpose [128,512]bf16 = 1.

Attention details: for key tile kt only queries 128*kt..128*kt+383 attend;
the valid band is r <= c <= r+256 for every kt -> one constant 0/1 bf16 mask
applied to P^T = exp(S^T/8) (logits are O(6): no row-max needed).  v^T is
transposed to v-natural on the PE and augmented with a ones column so the
softmax denominators fall out of the AV matmul (psum row 64).  The AV psum
is copied raw to SBUF immediately (numerator via DVE, denominator row via
the scalar engine to partition 0 - reciprocal_approx_fast misreads inputs at
a nonzero partition offset), freeing the single AV bank; reciprocal + gpsimd
partition_broadcast + multiply produce normalized U2 off the critical path.
"""

import os
from contextlib import ExitStack

import numpy as np

import concourse.bass as bass
import concourse.mybir as mybir
import concourse.tile as tile
from concourse import bacc
from concourse.bass_utils import run_bass_kernel_spmd
from concourse.masks import make_identity

F32 = mybir.dt.float32
N = 2048
D = 1024
HD = 64
WIN = 256
NPAIR = 2  # head pairs per core (4 heads)
NCH = D // 128  # 8 contraction chunks
NKT = N // 128  # 16 key tiles
SPAN = 384  # max query span per S^T key tile
QG = 512  # query group size
NQG = N // QG  # 4
SCALE = 1.0 / np.sqrt(HD)

_MM_DT_NAME = os.environ.get("BASS_MM_DT", "bfloat16")
MM_DT = getattr(mybir.dt, _MM_DT_NAME)

Exp = mybir.ActivationFunctionType.Exp
IS_GE = mybir.AluOpType.is_ge


def _span(kt):
    return min(SPAN, N - 128 * kt)


def _av_slices(kt):
    """For key tile kt return [(g, lo, hi, plo)]: query-group g consumes
    P^T[kt][:, lo:hi] into psum columns plo:plo+(hi-lo)."""
    span = _span(kt)
    out = []
    for g in range(NQG):
        lo = max(0, QG * g - 128 * kt)
        hi = min(span, QG * g + QG - 128 * kt)
        if lo < hi:
            out.append((g, lo, hi, 128 * kt + lo - QG * g))
    return out


def _group_kts(g):
    return [kt for kt in range(NKT) if any(s[0] == g for s in _av_slices(kt))]


def _s_kts(g):
    """Key tiles whose S^T is emitted in group g's stream (they need q/k
    columns up to 512*g+512, i.e. group <= g)."""
    if g < NQG:
        return [kt for kt in (4 * g - 2, 4 * g - 1, 4 * g, 4 * g + 1) if kt >= 0]
    return [14, 15]


def _emit(ctx: ExitStack, tc: tile.TileContext, aps, mm_dt, debug_taps=None):
    nc = tc.nc
    xTd, wq, wk, wv, wo, bq, bk, bv, out = aps
    MDT = mm_dt

    consts = ctx.enter_context(tc.tile_pool(name="consts", bufs=1))
    persist = ctx.enter_context(tc.tile_pool(name="persist", bufs=1))

    ident_f = consts.tile([128, 128], F32, tag="ident_f")
    make_identity(nc, ident_f)
    ident = consts.tile([128, 128], MDT, tag="ident")
    nc.vector.tensor_copy(ident, ident_f)

    mask_f = consts.tile([128, SPAN], F32, tag="mask_f")
    nc.gpsimd.memset(mask_f, 1.0)
    nc.gpsimd.affine_select(
        out=mask_f, in_=mask_f, compare_op=IS_GE, fill=0.0,
        base=0, pattern=[[1, SPAN]], channel_multiplier=-1,
    )  # keep c - r >= 0
    nc.gpsimd.affine_select(
        out=mask_f, in_=mask_f, compare_op=IS_GE, fill=0.0,
        base=WIN, pattern=[[-1, SPAN]], channel_multiplier=1,
    )  # keep r - c + WIN >= 0
    mask2 = consts.tile([128, 2, SPAN], MDT, tag="mask2")
    nc.vector.tensor_copy(mask2[:, 0, :], mask_f)
    nc.vector.tensor_copy(mask2[:, 1, :], mask_f)

    onesf = consts.tile([128, 1], F32, tag="onesf")
    nc.vector.memset(onesf, 1.0)
    zf = consts.tile([1, QG], F32, tag="zf")
    nc.vector.memset(zf, 0.0)
    zcol = consts.tile([1, 65], MDT, tag="zcol")
    nc.vector.tensor_copy(zcol, zf[:, 0:65])
    zrow = consts.tile([1, QG], MDT, tag="zrow")
    nc.vector.tensor_copy(zrow, zf)

    qT = persist.tile([128, NPAIR, N], MDT, tag="qT")
    kT = persist.tile([128, NPAIR, N], MDT, tag="kT")
    vaug = [persist.tile([128, NKT, 2, HD + 1], MDT, tag=f"vaug{p}",
                         name=f"vaug{p}")
            for p in range(NPAIR)]
    U2 = persist.tile([128, NPAIR, N], MDT, tag="U2")

    # ---- input DMAs: xT group 0 first on the sync queue; weights/biases on
    # the scalar engine's queue so their descriptor issue doesn't delay xT.
    xT_sb = consts.tile([128, NCH, N], MDT, tag="xT_sb")
    nc.sync.dma_start(out=xT_sb[:, :, 0:QG], in_=xTd[:, 0, :, :])
    w_sb = {}
    b_sb = {}
    for nm, wap in (("q", wq), ("k", wk), ("v", wv)):
        t = consts.tile([128, NCH, 2 * 128], MDT, tag=f"w{nm}", name=f"w{nm}")
        nc.scalar.dma_start(out=t, in_=wap)
        w_sb[nm] = t
    for nm, bap in (("q", bq), ("k", bk), ("v", bv)):
        t = consts.tile([128, NPAIR], F32, tag=f"b{nm}", name=f"b{nm}")
        nc.scalar.dma_start(out=t, in_=bap)
        b_sb[nm] = t
    wo_sb = consts.tile([128, NPAIR, D], MDT, tag="wo")
    nc.scalar.dma_start(out=wo_sb, in_=wo)

    acc = ctx.enter_context(tc.tile_pool(name="acc", bufs=2, space="PSUM"))
    psS = ctx.enter_context(tc.tile_pool(name="psS", bufs=2, space="PSUM"))
    psAV = ctx.enter_context(tc.tile_pool(name="psAV", bufs=1, space="PSUM"))
    psT = ctx.enter_context(tc.tile_pool(name="psT", bufs=1, space="PSUM"))
    vt_pool = ctx.enter_context(tc.tile_pool(name="vt", bufs=2))
    pt_pool = ctx.enter_context(tc.tile_pool(name="pt", bufs=16))
    u2r_pool = ctx.enter_context(tc.tile_pool(name="u2r", bufs=3))
    rb_pool = ctx.enter_context(tc.tile_pool(name="rb", bufs=3))
    ob_pool = ctx.enter_context(tc.tile_pool(name="ob", bufs=4))

    for pair in range(NPAIR):
        nc.vector.tensor_copy(
            vaug[pair][:, :, :, HD:HD + 1],
            onesf.broadcast_to((128, NKT, 2, 1)),
        )

    pts = {}
    vts = {}
    nblk = [0]  # out-proj block counter (for cast-engine alternation)

    def qkv_chain(g, pair, nm):
        gsl = slice(QG * g, QG * (g + 1))
        psq = acc.tile([128, QG], F32, tag="acc")
        for c in range(NCH):
            nc.tensor.matmul(
                psq,
                w_sb[nm][:, c, 128 * pair:128 * (pair + 1)],
                xT_sb[:, c, gsl],
                start=(c == 0), stop=(c == NCH - 1),
            )
        if nm == "v":
            vt = vts[g]
            dst = vt[:, pair, :]
        else:
            dst = (qT if nm == "q" else kT)[:, pair, gsl]
        nc.vector.tensor_scalar_add(dst, psq, b_sb[nm][:, pair:pair + 1])

    def vtrans(g, pair):
        vt = vts[g]
        pst = psT.tile([128, QG], MDT, tag="pst")
        for j in range(4):
            nc.tensor.transpose(
                pst[:, 128 * j:128 * (j + 1)],
                vt[:, pair, 128 * j:128 * (j + 1)], ident,
            )
        nc.vector.tensor_copy(
            vaug[pair][:, 4 * g:4 * (g + 1), :, 0:HD],
            pst.rearrange("p (j h d) -> p j h d", j=4, h=2),
        )

    def s_unit(kt, pair):
        span = _span(kt)
        q0 = 128 * kt
        pss = psS.tile([128, 2, QG], F32, tag="pss")
        for h in range(2):
            hb = 64 * h
            nc.tensor.matmul(
                pss[:, h, 0:span],
                kT[hb:hb + 64, pair, q0:q0 + 128],
                qT[hb:hb + 64, pair, q0:q0 + span],
                start=True, stop=True,
            )
        pt = pt_pool.tile([128, 2, SPAN], MDT, tag="pt")
        nc.scalar.activation(
            pt[:, :, 0:span], pss[:, :, 0:span], Exp, scale=SCALE)
        nc.vector.tensor_mul(
            pt[:, :, 0:128], pt[:, :, 0:128], mask2[:, :, 0:128])
        if span > WIN:
            nc.vector.tensor_mul(
                pt[:, :, WIN:span], pt[:, :, WIN:span],
                mask2[:, :, WIN:span])
        if debug_taps is not None and pair == 0 and kt < 4:
            nc.sync.dma_start(out=debug_taps[0][:, kt, :, :], in_=pt)
        pts[(pair, kt)] = pt

    def av_unit(g, pair, h):
        psav = psAV.tile([65, QG], F32, tag="psav")
        nc.tensor.matmul(psav, zcol, zrow, start=True, stop=False)
        kts = _group_kts(g)
        for i, kt in enumerate(kts):
            (lo, hi, plo) = next(
                (s[1], s[2], s[3]) for s in _av_slices(kt) if s[0] == g)
            nc.tensor.matmul(
                psav[:, plo:plo + (hi - lo)],
                vaug[pair][:, kt, h, :],
                pts[(pair, kt)][:, h, lo:hi],
                start=False, stop=(i == len(kts) - 1),
            )
        u2rn = u2r_pool.tile([64, QG], F32, tag="u2rn")
        nc.vector.tensor_copy(u2rn, psav[0:64, :])
        rt0 = rb_pool.tile([1, QG], F32, tag="rt0")
        nc.scalar.copy(rt0, psav[64:65, :])
        rtmp = rb_pool.tile([1, QG], F32, tag="rtmp")
        nc.vector.reciprocal_approx_fast(out=rtmp, in_=rt0)
        rbt = rb_pool.tile([64, QG], F32, tag="rbt")
        nc.gpsimd.partition_broadcast(rbt, rtmp)
        nc.vector.tensor_mul(
            U2[64 * h:64 * (h + 1), pair, QG * g:QG * (g + 1)],
            u2rn, rbt,
        )

    def o_block(g, qt, dh):
        pso = acc.tile([128, QG], F32, tag="acc")
        for pair in range(NPAIR):
            nc.tensor.matmul(
                pso,
                U2[:, pair, 128 * qt:128 * (qt + 1)],
                wo_sb[:, pair, QG * dh:QG * (dh + 1)],
                start=(pair == 0), stop=(pair == NPAIR - 1),
            )
        ob = ob_pool.tile([128, QG], MDT, tag="ob")
        if nblk[0] % 2 == 0:
            nc.scalar.copy(ob, pso)
        else:
            nc.vector.tensor_copy(ob, pso)
        nblk[0] += 1
        nc.sync.dma_start(
            out=out[128 * qt:128 * (qt + 1), QG * dh:QG * (dh + 1)],
            in_=ob,
        )

    def o_blocks(g):
        return [(g, 4 * g + i, dh) for i in range(4) for dh in range(2)]

    # ---------------- software-pipelined main loop ----------------
    for g in range(NQG):
        if g + 1 < NQG:
            nc.sync.dma_start(
                out=xT_sb[:, :, QG * (g + 1):QG * (g + 2)],
                in_=xTd[:, g + 1, :, :])
        vts[g] = vt_pool.tile([128, NPAIR, QG], MDT, tag="vt", name="vt")
        obl = o_blocks(g - 2) if g >= 2 else []
        for i, (pair, nm) in enumerate(((0, "q"), (1, "q"), (0, "k"), (1, "k"))):
            qkv_chain(g, pair, nm)
            if obl:
                o_block(*obl[2 * i])
                o_block(*obl[2 * i + 1])
        kts = _s_kts(g)
        if g == 0:
            for kt in kts:
                s_unit(kt, 0)
                s_unit(kt, 1)
            qkv_chain(g, 0, "v")
            qkv_chain(g, 1, "v")
            vtrans(g, 0)
            vtrans(g, 1)
        else:
            s_unit(kts[0], 0)
            s_unit(kts[0], 1)
            s_unit(kts[1], 0)
            s_unit(kts[1], 1)
            qkv_chain(g, 0, "v")
            av_unit(g - 1, 0, 0)
            qkv_chain(g, 1, "v")
            av_unit(g - 1, 0, 1)
            s_unit(kts[2], 0)
            s_unit(kts[2], 1)
            av_unit(g - 1, 1, 0)
            vtrans(g, 0)
            vtrans(g, 1)
            av_unit(g - 1, 1, 1)
            s_unit(kts[3], 0)
            s_unit(kts[3], 1)

    # ---------------- drain ----------------
    obl = o_blocks(2)
    s_unit(14, 0)
    s_unit(14, 1)
    o_block(*obl[0])
    o_block(*obl[1])
    s_unit(15, 0)
    s_unit(15, 1)
    for b_ in obl[2:6]:
        o_block(*b_)
    av_unit(3, 0, 0)
    o_block(*obl[6])
    av_unit(3, 0, 1)
    o_block(*obl[7])
    av_unit(3, 1, 0)
    av_unit(3, 1, 1)
    for b_ in o_blocks(3):
        o_block(*b_)
    if debug_taps is not None:
        nc.sync.dma_start(out=debug_taps[1], in_=U2)


def build(mm_dt=MM_DT):
    nc = bacc.Bacc("TRN2", target_bir_lowering=False, debug=False)
    xT = nc.dram_tensor("xT", [128, NQG, NCH, QG], mm_dt, kind="ExternalInput").ap()
    wq = nc.dram_tensor("wq", [128, NCH, 256], mm_dt, kind="ExternalInput").ap()
    wk = nc.dram_tensor("wk", [128, NCH, 256], mm_dt, kind="ExternalInput").ap()
    wv = nc.dram_tensor("wv", [128, NCH, 256], mm_dt, kind="ExternalInput").ap()
    wo = nc.dram_tensor("wo", [128, NPAIR, D], mm_dt, kind="ExternalInput").ap()
    bq = nc.dram_tensor("bq", [128, NPAIR], F32, kind="ExternalInput").ap()
    bk = nc.dram_tensor("bk", [128, NPAIR], F32, kind="ExternalInput").ap()
    bv = nc.dram_tensor("bv", [128, NPAIR], F32, kind="ExternalInput").ap()
    out = nc.dram_tensor("out", [N, D], mm_dt, kind="ExternalOutput").ap()
    with tile.TileContext(nc) as tc, ExitStack() as ctx:
        _emit(ctx, tc, (xT, wq, wk, wv, wo, bq, bk, bv, out), mm_dt)
    nc.compile()
    return nc


def shard_inputs(x, Wq, bq, Wk, bk, Wv, bv, Wo, bo):
    """Full inputs -> 8 per-core input maps, pre-arranged so every DMA line
    is contiguous per partition."""
    mdt = mybir.dt.np(MM_DT)

    def warr(W, cs):  # [1024, 256] -> [128, 8, 256]
        return np.ascontiguousarray(
            W[:, cs].reshape(NCH, 128, 256).transpose(1, 0, 2)).astype(mdt)

    xTb = [np.ascontiguousarray(
        x[b].T.reshape(NCH, 128, NQG, QG).transpose(1, 2, 0, 3)).astype(mdt)
        for b in range(2)]
    in_maps = []
    for c in range(8):
        b, hg = c // 4, c % 4
        cs = slice(256 * hg, 256 * (hg + 1))
        in_maps.append({
            "xT": xTb[b],
            "wq": warr(Wq, cs),
            "wk": warr(Wk, cs),
            "wv": warr(Wv, cs),
            "wo": np.ascontiguousarray(
                Wo[cs, :].reshape(NPAIR, 128, D).transpose(1, 0, 2)).astype(mdt),
            "bq": np.ascontiguousarray(bq[cs].reshape(NPAIR, 128).T),
            "bk": np.ascontiguousarray(bk[cs].reshape(NPAIR, 128).T),
            "bv": np.ascontiguousarray(bv[cs].reshape(NPAIR, 128).T),
        })
    return in_maps


def assemble(results, bo):
    outs = [np.asarray(r["out"], dtype=np.float32) for r in results]
    full = np.empty((2, N, D), dtype=np.float32)
    for b in range(2):
        full[b] = outs[4 * b] + outs[4 * b + 1] + outs[4 * b + 2] + outs[4 * b + 3]
        full[b] += bo[None, :]
    return full


_NC_CACHE = {}


def _get_nc():
    key = _MM_DT_NAME
    if key not in _NC_CACHE:
        _NC_CACHE[key] = build()
    return _NC_CACHE[key]


def kernel(x, Wq, bq, Wk, bk, Wv, bv, Wo, bo, _trace=False):
    x, Wq, bq, Wk, bk, Wv, bv, Wo, bo = (
        np.asarray(a, dtype=np.float32)
        for a in (x, Wq, bq, Wk, bk, Wv, bv, Wo, bo)
    )
    nc = _get_nc()
    in_maps = shard_inputs(x, Wq, bq, Wk, bk, Wv, bv, Wo, bo)
    res = run_bass_kernel_spmd(nc, in_maps, core_ids=list(range(8)), trace=_trace)
    full = assemble(res.results, bo)
    if _trace:
        kernel.last_result = res
    return full
